# revision 1
# baseline (speedup 1.0000x reference)
"""MoE (top-2 of 8 experts, D=H=1024) on 8 Trainium2 NeuronCores.

Strategy (expert-parallel, matching the sharding hint):
  - Host computes the router (softmax + top-k + expert-sort dispatch) in
    float64 -- the dispatch/sharding decision, 0.2% of total FLOPs.
  - Tokens are gathered per expert (capacity-padded); core c gets expert c's
    token block plus expert c's weights.
  - Each core runs the 2-layer expert MLP in "transposed activation" layout
    (activations are [feature, token]) so no on-device transposes are needed:
        hT = w_in.T @ xT   (lhsT = w_in chunk, natural layout)
        yT = w_out.T @ hT  (lhsT = w_out chunk, natural layout)
    Loops are k-outer so matmuls start as soon as the first weight/activation
    chunks land, with 8 concurrent PSUM accumulation groups per layer.
  - Host scales rows by the gate probability (zero for padding rows) and
    scatter-adds back into the [T, D] output.
"""

import os
import sys

import numpy as np

for _p in ("/opt/trn_rl_repo", "/root/.axon_site/_ro/trn_rl_repo"):
    if os.path.isdir(_p) and _p not in sys.path:
        sys.path.append(_p)


def _ensure_ntff_hook():
    """Register the axon NTFF profiling hook if the image's antenv lacks it."""
    try:
        import antenv.axon_hooks  # noqa: F401

        return
    except ImportError:
        pass
    import types

    try:
        import antenv
    except ImportError:
        return
    mod = types.ModuleType("antenv.axon_hooks")
    _hook = [None]
    mod.set_axon_ntff_profile_hook = lambda h: _hook.__setitem__(0, h)
    mod.get_axon_ntff_profile_hook = lambda: _hook[0]
    sys.modules["antenv.axon_hooks"] = mod
    antenv.axon_hooks = mod
    try:
        from trn_agent_boot.trn_boot import _ntff_profile_via_ctypes

        mod.set_axon_ntff_profile_hook(
            _ntff_profile_via_ctypes("/opt/axon/libaxon_pjrt.so")
        )
    except Exception:
        pass


_ensure_ntff_hook()

D, H, E, TOPK = 1024, 1024, 8, 2
N_CORES = 8
P = 128  # partitions

# Matmul input dtype: float32 (exact, 4 cyc/row), float32r (1 cyc/row,
# ~13-bit multiplies), float16 / bfloat16 (1 cyc/row + fast weight load,
# half the weight DMA bytes).
MM_DTYPE = os.environ.get("MOE_MM_DTYPE", "float16")

_compiled_cache = {}


def _np_mm_dtype(mm_dtype_str):
    if mm_dtype_str in ("float32", "float32r"):
        return np.float32
    if mm_dtype_str == "float16":
        return np.float16
    if mm_dtype_str == "bfloat16":
        import ml_dtypes

        return np.dtype(ml_dtypes.bfloat16)
    raise ValueError(mm_dtype_str)


def _build_program(C, mm_dtype_str):
    """One expert's MLP over a [C] token block; same program on all cores."""
    from concourse import bacc, mybir, tile

    f32 = mybir.dt.float32
    mm_dt = getattr(mybir.dt, mm_dtype_str)
    nc = bacc.Bacc(None, target_bir_lowering=False, debug=False)

    xT_d = nc.dram_tensor("xT", [D, C], mm_dt, kind="ExternalInput")
    w_in_d = nc.dram_tensor("w_in", [D, H], mm_dt, kind="ExternalInput")
    w_out_d = nc.dram_tensor("w_out", [H, D], mm_dt, kind="ExternalInput")
    bias_d = nc.dram_tensor("bias", [2, H], f32, kind="ExternalInput")
    yT_d = nc.dram_tensor("yT", [D, C], f32, kind="ExternalOutput")

    KD = D // P  # contraction chunks, layer 1 (and output chunks, layer 2)
    KH = H // P

    with tile.TileContext(nc) as tc:
        with (
            tc.tile_pool(name="wpool", bufs=1) as wpool,
            tc.tile_pool(name="xpool", bufs=1) as xpool,
            tc.tile_pool(name="hpool", bufs=1) as hpool,
            tc.tile_pool(name="ypool", bufs=1) as ypool,
            tc.tile_pool(name="bpool", bufs=1) as bpool,
            tc.tile_pool(name="psum", bufs=4, space="PSUM") as pspool,
        ):
            w1 = wpool.tile([P, KD, H], mm_dt, tag="w1")
            xt = xpool.tile([P, KD, C], mm_dt, tag="xt")
            w2 = wpool.tile([P, KH, D], mm_dt, tag="w2")
            bias = bpool.tile([P, 2, KH], f32, tag="bias")

            xT_r = xT_d.rearrange("(k p) c -> p k c", p=P)
            # Single SP HWDGE ring, FIFO completion, issued in consumption
            # order.  dma_start costs ~650ns of sequencer time regardless of
            # size, so use small chunks up front (compute starts early) and
            # big chunks later (few issues; transfers pace at HBM rate).
            nc.scalar.dma_start(bias[:], bias_d.rearrange("b (m p) -> p b m", p=P))

            def w_chunk(dst, src, k0, k1):
                src_r = src.rearrange("(k p) h -> p k h", p=P)
                nc.sync.dma_start(dst[:, k0:k1, :], src_r[:, k0:k1, :])

            w_chunk(w1, w_in_d, 0, 1)
            nc.sync.dma_start(xt[:, 0:2, :], xT_r[:, 0:2, :])
            w_chunk(w1, w_in_d, 1, 2)
            w_chunk(w1, w_in_d, 2, 4)
            nc.sync.dma_start(xt[:, 2:KD, :], xT_r[:, 2:KD, :])
            w_chunk(w1, w_in_d, 4, 8)
            w_chunk(w2, w_out_d, 0, 4)
            w_chunk(w2, w_out_d, 4, 8)

            gelu = mybir.ActivationFunctionType.Gelu_apprx_tanh

            # PE warmup during the initial DMA window: ~3us of dummy matmuls
            # flips the HAM clock gate to 8/8 before the real stream begins.
            wz = bpool.tile([P, P], mm_dt, tag="wz")
            nc.vector.memset(wz[:], 0.0)
            psw = pspool.tile([P, 2, 512], f32, tag="ps", name="ps_warm")
            NWARM = 36
            for i in range(NWARM):
                nc.tensor.matmul(
                    psw[:, 0, :P], wz[:], wz[:], start=(i == 0), stop=(i == NWARM - 1)
                )

            # layer 1, k-outer: 8 concurrent accumulation groups (one/bank)
            ht = hpool.tile([P, KH, C], mm_dt, tag="ht")
            ps1 = [pspool.tile([P, 2, 512], f32, tag="ps", name=f"ps1_{i}") for i in range(KH // 2)]
            for k in range(KD):
                for m in range(KH):
                    nc.tensor.matmul(
                        ps1[m // 2][:, m % 2, :C],
                        w1[:, k, m * P : (m + 1) * P],
                        xt[:, k, :],
                        start=(k == 0),
                        stop=(k == KD - 1),
                    )
            for m in range(KH):
                nc.scalar.activation(
                    ht[:, m, :],
                    ps1[m // 2][:, m % 2, :C],
                    gelu,
                    bias=bias[:, 0, m : m + 1],
                )

            # layer 2, k-outer
            yt = ypool.tile([P, KD, C], f32, tag="yt")
            ps2 = [pspool.tile([P, 2, 512], f32, tag="ps", name=f"ps2_{i}") for i in range(KD // 2)]
            for k in range(KH):
                for m in range(KD):
                    nc.tensor.matmul(
                        ps2[m // 2][:, m % 2, :C],
                        w2[:, k, m * P : (m + 1) * P],
                        ht[:, k, :],
                        start=(k == 0),
                        stop=(k == KH - 1),
                    )
            # PSUM -> SBUF via plain DVE copies (fastest PSUM drain); the
            # host adds b_out and the gate-probability scaling during the
            # scatter-combine
            yT_r = yT_d.rearrange("(m p) c -> p m c", p=P)
            for j in range(KD // 2):
                nc.vector.tensor_copy(
                    yt[:, 2 * j : 2 * j + 2, :], ps2[j][:, :, :C]
                )
                nc.scalar.dma_start(
                    yT_r[:, 2 * j : 2 * j + 2, :], yt[:, 2 * j : 2 * j + 2, :]
                )

    nc.compile()
    if not nc.is_finalized():
        nc.finalize()
    return nc


def _get_program(C):
    key = (C, MM_DTYPE)
    if key not in _compiled_cache:
        _compiled_cache[key] = _build_program(C, MM_DTYPE)
    return _compiled_cache[key]


def _route(x2, router_w):
    """Host router in float64: top-2 experts + gate probs per token."""
    logits = x2.astype(np.float64) @ np.asarray(router_w, np.float64)
    logits -= logits.max(axis=-1, keepdims=True)
    ex = np.exp(logits)
    probs = ex / ex.sum(axis=-1, keepdims=True)
    top_e = np.argsort(-probs, axis=-1, kind="stable")[:, :TOPK]  # [T, K]
    top_p = np.take_along_axis(probs, top_e, axis=-1)  # [T, K]
    return top_e, top_p.astype(np.float32)


def kernel(input_batch, router_w, w_in, b_in, w_out, b_out, run_kwargs=None):
    from concourse.bass_utils import run_bass_kernel_spmd

    x = np.ascontiguousarray(np.asarray(input_batch, np.float32))
    B, S, Dm = x.shape
    T = B * S
    x2 = x.reshape(T, Dm)

    top_e, top_p = _route(x2, router_w)

    # per-expert dispatch lists, in expert-sorted (token, k) order like the
    # reference's stable argsort over flattened (token, k) pairs
    tok_lists = [[] for _ in range(E)]
    p_lists = [[] for _ in range(E)]
    for t in range(T):
        for j in range(TOPK):
            e = top_e[t, j]
            tok_lists[e].append(t)
            p_lists[e].append(top_p[t, j])

    counts = [len(l) for l in tok_lists]
    # capacity per wave; a PSUM bank caps the matmul free dim at 512, so an
    # expert with >512 routed tokens (never happens for the spec'd input
    # distribution) is processed in multiple SPMD waves
    n_waves = max(1, -(-max(counts) // 512))
    if n_waves == 1:
        C = max(256, -(-max(counts) // 8) * 8)  # multiple of 8
    else:
        C = 512

    nc = _get_program(C)
    mm_np = _np_mm_dtype(MM_DTYPE)

    w_in = np.asarray(w_in, np.float32)
    w_out = np.asarray(w_out, np.float32)
    b_in = np.asarray(b_in, np.float32)
    b_out = np.asarray(b_out, np.float32)

    out = np.zeros((T, Dm), np.float32)
    for w in range(n_waves):
        in_maps = []
        for e in range(E):
            idx = np.asarray(tok_lists[e][w * C : (w + 1) * C], np.int64)
            xT = np.zeros((D, C), mm_np)
            if len(idx):
                xT[:, : len(idx)] = x2[idx].T.astype(mm_np)
            in_maps.append(
                {
                    "xT": xT,
                    "w_in": np.ascontiguousarray(w_in[e]).astype(mm_np),
                    "w_out": np.ascontiguousarray(w_out[e]).astype(mm_np),
                    "bias": np.stack([b_in[e], b_out[e]]),
                }
            )

        res = run_bass_kernel_spmd(
            nc, in_maps, core_ids=list(range(N_CORES)), **(run_kwargs or {})
        )
        kernel.last_results = res

        for e in range(E):
            idx = np.asarray(tok_lists[e][w * C : (w + 1) * C], np.int64)
            n = len(idx)
            if n == 0:
                continue
            p = np.asarray(p_lists[e][w * C : (w + 1) * C], np.float32)
            y = (res.results[e]["yT"][:, :n].T + b_out[e]) * p[:, None]
            np.add.at(out, idx, y)

    return out.reshape(B, S, Dm)



# revision 2
# speedup vs baseline: 1.0421x; 1.0421x over previous
"""MoE (top-2 of 8 experts, D=H=1024) on 8 Trainium2 NeuronCores.

Strategy (expert-parallel, matching the sharding hint):
  - Host computes the router (softmax + top-k + expert-sort dispatch) in
    float64 -- the dispatch/sharding decision, 0.2% of total FLOPs.
  - Tokens are gathered per expert (capacity-padded); core c gets expert c's
    token block plus expert c's weights.
  - Each core runs the 2-layer expert MLP in "transposed activation" layout
    (activations are [feature, token]) so no on-device transposes are needed.
    Both layers are m-outer: each 128-wide output block accumulates over all
    contraction chunks in a single PSUM bank, then is drained (gelu for layer
    1, fp16 copy + DMA-out for layer 2) while the PE works on the next block.
    Host pre-reorders the weights so each m-block is one contiguous DMA chunk
    and availability tracks the stream.
  - DMA: weights stream on the Sync HWDGE ring in consumption order (FIFO =
    priority); the Scalar ring carries the bias + first w1 chunk up front and
    the output chunks at the end.
  - Host scales rows by the gate probability (zero for padding rows) and
    scatter-adds back into the [T, D] output.
"""

import os
import sys

import numpy as np

for _p in ("/opt/trn_rl_repo", "/root/.axon_site/_ro/trn_rl_repo"):
    if os.path.isdir(_p) and _p not in sys.path:
        sys.path.append(_p)


def _ensure_ntff_hook():
    """Register the axon NTFF profiling hook if the image's antenv lacks it."""
    try:
        import antenv.axon_hooks  # noqa: F401

        return
    except ImportError:
        pass
    import types

    try:
        import antenv
    except ImportError:
        return
    mod = types.ModuleType("antenv.axon_hooks")
    _hook = [None]
    mod.set_axon_ntff_profile_hook = lambda h: _hook.__setitem__(0, h)
    mod.get_axon_ntff_profile_hook = lambda: _hook[0]
    sys.modules["antenv.axon_hooks"] = mod
    antenv.axon_hooks = mod
    try:
        from trn_agent_boot.trn_boot import _ntff_profile_via_ctypes

        mod.set_axon_ntff_profile_hook(
            _ntff_profile_via_ctypes("/opt/axon/libaxon_pjrt.so")
        )
    except Exception:
        pass


_ensure_ntff_hook()

D, H, E, TOPK = 1024, 1024, 8, 2
N_CORES = 8
P = 128  # partitions
KD = D // P
KH = H // P

NWARM = int(os.environ.get("MOE_NWARM", "30"))

_compiled_cache = {}


def _build_program(C):
    """One expert's MLP over a [C] token block; same program on all cores."""
    from concourse import bacc, mybir, tile

    f32 = mybir.dt.float32
    f16 = mybir.dt.float16
    nc = bacc.Bacc(None, target_bir_lowering=False, debug=False)

    # Host-packed layouts: everything contiguous per partition.
    #   xt[p, k, c]    = x[token c, k*128+p]
    #   w1[p, m, k, j] = w_in[k*128+p, m*128+j]
    #   w2[p, m, k, j] = w_out[k*128+p, m*128+j]
    #   b1[p, m]       = b_in[m*128+p]
    #   yT[p, m, c]    = y[token c, m*128+p]
    xt_d = nc.dram_tensor("xt", [P, KD, C], f16, kind="ExternalInput")
    w1_d = nc.dram_tensor("w1", [P, KH, KD, P], f16, kind="ExternalInput")
    w2_d = nc.dram_tensor("w2", [P, KD, KH, P], f16, kind="ExternalInput")
    b1_d = nc.dram_tensor("b1", [P, KH], f32, kind="ExternalInput")
    yT_d = nc.dram_tensor("yT", [P, KD, C], f16, kind="ExternalOutput")

    with tile.TileContext(nc) as tc:
        with (
            tc.tile_pool(name="wpool", bufs=1) as wpool,
            tc.tile_pool(name="xpool", bufs=1) as xpool,
            tc.tile_pool(name="hpool", bufs=1) as hpool,
            tc.tile_pool(name="ypool", bufs=1) as ypool,
            tc.tile_pool(name="bpool", bufs=1) as bpool,
            tc.tile_pool(name="ps1pool", bufs=4, space="PSUM") as ps1pool,
            tc.tile_pool(name="ps2pool", bufs=4, space="PSUM") as ps2pool,
        ):
            w1 = wpool.tile([P, KH, KD, P], f16, tag="w1")
            w2 = wpool.tile([P, KD, KH, P], f16, tag="w2")
            xt = xpool.tile([P, KD, C], f16, tag="xt")
            b1 = bpool.tile([P, KH], f32, tag="b1")
            ht = hpool.tile([P, KH, C], f16, tag="ht")
            yt = ypool.tile([P, KD, C], f16, tag="yt")

            # Scalar HWDGE ring: bias + first w1 chunk (first-needed).
            nc.scalar.dma_start(b1[:], b1_d[:])
            nc.scalar.dma_start(w1[:, 0:2], w1_d[:, 0:2])
            # Sync HWDGE ring, FIFO = priority order: activations first,
            # then remaining w1 chunks, then w2.
            nc.sync.dma_start(xt[:], xt_d[:])
            nc.sync.dma_start(w1[:, 2:4], w1_d[:, 2:4])
            nc.sync.dma_start(w1[:, 4:6], w1_d[:, 4:6])
            nc.sync.dma_start(w1[:, 6:8], w1_d[:, 6:8])
            nc.sync.dma_start(w2[:, 0:4], w2_d[:, 0:4])
            nc.sync.dma_start(w2[:, 4:8], w2_d[:, 4:8])

            gelu = mybir.ActivationFunctionType.Gelu_apprx_tanh

            # PE warmup during the initial DMA window: dummy matmuls flip the
            # HAM clock gate to 8/8 before the real stream begins.
            wz = bpool.tile([P, P], f16, tag="wz")
            nc.vector.memset(wz[:], 0.0)
            psw = ps2pool.tile([P, 512], f32, tag="ps2", name="ps_warm")
            for i in range(NWARM):
                nc.tensor.matmul(
                    psw[:, :P], wz[:], wz[:], start=(i == 0), stop=(i == NWARM - 1)
                )

            # layer 1, m-outer: one PSUM bank per output block, gelu drains
            # while the PE streams the next block.
            ps1 = [ps1pool.tile([P, 512], f32, tag="ps1", name=f"ps1_{m}") for m in range(KH)]
            for m in range(KH):
                for k in range(KD):
                    nc.tensor.matmul(
                        ps1[m][:, :C],
                        w1[:, m, k, :],
                        xt[:, k, :],
                        start=(k == 0),
                        stop=(k == KD - 1),
                    )
                nc.scalar.activation(
                    ht[:, m, :], ps1[m][:, :C], gelu, bias=b1[:, m : m + 1]
                )

            # layer 2, m-outer: drain each output block to SBUF (fp16) and
            # DMA it out (Scalar ring) while the next block computes.
            ps2 = [ps2pool.tile([P, 512], f32, tag="ps2", name=f"ps2_{m}") for m in range(KD)]
            for m in range(KD):
                for k in range(KH):
                    nc.tensor.matmul(
                        ps2[m][:, :C],
                        w2[:, m, k, :],
                        ht[:, k, :],
                        start=(k == 0),
                        stop=(k == KH - 1),
                    )
                nc.vector.tensor_copy(yt[:, m, :], ps2[m][:, :C])
                if m % 2 == 1:
                    nc.scalar.dma_start(
                        yT_d[:, m - 1 : m + 1, :], yt[:, m - 1 : m + 1, :]
                    )

    nc.compile()
    if not nc.is_finalized():
        nc.finalize()
    return nc


def _get_program(C):
    if C not in _compiled_cache:
        _compiled_cache[C] = _build_program(C)
    return _compiled_cache[C]


def _route(x2, router_w):
    """Host router in float64: top-2 experts + gate probs per token."""
    logits = x2.astype(np.float64) @ np.asarray(router_w, np.float64)
    logits -= logits.max(axis=-1, keepdims=True)
    ex = np.exp(logits)
    probs = ex / ex.sum(axis=-1, keepdims=True)
    top_e = np.argsort(-probs, axis=-1, kind="stable")[:, :TOPK]  # [T, K]
    top_p = np.take_along_axis(probs, top_e, axis=-1)  # [T, K]
    return top_e, top_p.astype(np.float32)


def _pack_w(w):
    """[D, H] -> [P, KH, KD, P] fp16 with w_packed[p, m, k, j] = w[k*128+p, m*128+j]."""
    return np.ascontiguousarray(
        w.reshape(KD, P, KH, P).transpose(1, 2, 0, 3)
    ).astype(np.float16)


def kernel(input_batch, router_w, w_in, b_in, w_out, b_out, run_kwargs=None):
    from concourse.bass_utils import run_bass_kernel_spmd

    x = np.ascontiguousarray(np.asarray(input_batch, np.float32))
    B, S, Dm = x.shape
    T = B * S
    x2 = x.reshape(T, Dm)

    top_e, top_p = _route(x2, router_w)

    # per-expert dispatch lists, in expert-sorted (token, k) order like the
    # reference's stable argsort over flattened (token, k) pairs
    tok_lists = [[] for _ in range(E)]
    p_lists = [[] for _ in range(E)]
    for t in range(T):
        for j in range(TOPK):
            e = top_e[t, j]
            tok_lists[e].append(t)
            p_lists[e].append(top_p[t, j])

    counts = [len(l) for l in tok_lists]
    # capacity per wave; a PSUM bank caps the matmul free dim at 512, so an
    # expert with >512 routed tokens (never happens for the spec'd input
    # distribution) is processed in multiple SPMD waves
    n_waves = max(1, -(-max(counts) // 512))
    if n_waves == 1:
        C = max(256, -(-max(counts) // 8) * 8)  # multiple of 8
    else:
        C = 512

    nc = _get_program(C)

    w_in = np.asarray(w_in, np.float32)
    w_out = np.asarray(w_out, np.float32)
    b_in = np.asarray(b_in, np.float32)
    b_out = np.asarray(b_out, np.float32)

    w1_packed = [_pack_w(w_in[e]) for e in range(E)]
    w2_packed = [_pack_w(w_out[e]) for e in range(E)]
    b1_packed = [
        np.ascontiguousarray(b_in[e].reshape(KH, P).T).astype(np.float32)
        for e in range(E)
    ]

    out = np.zeros((T, Dm), np.float32)
    for w in range(n_waves):
        in_maps = []
        for e in range(E):
            idx = np.asarray(tok_lists[e][w * C : (w + 1) * C], np.int64)
            xt = np.zeros((P, KD, C), np.float16)
            if len(idx):
                # xt[p, k, c] = x2[idx[c], k*128+p]
                xt[:, :, : len(idx)] = (
                    x2[idx].astype(np.float16).T.reshape(KD, P, len(idx)).transpose(1, 0, 2)
                )
            in_maps.append(
                {
                    "xt": xt,
                    "w1": w1_packed[e],
                    "w2": w2_packed[e],
                    "b1": b1_packed[e],
                }
            )

        res = run_bass_kernel_spmd(
            nc, in_maps, core_ids=list(range(N_CORES)), **(run_kwargs or {})
        )
        kernel.last_results = res

        for e in range(E):
            idx = np.asarray(tok_lists[e][w * C : (w + 1) * C], np.int64)
            n = len(idx)
            if n == 0:
                continue
            p = np.asarray(p_lists[e][w * C : (w + 1) * C], np.float32)
            yT = res.results[e]["yT"]  # [P, KD, C] fp16
            y = yT.transpose(2, 1, 0).reshape(C, Dm)[:n].astype(np.float32)
            y = (y + b_out[e]) * p[:, None]
            np.add.at(out, idx, y)

    return out.reshape(B, S, Dm)


# revision 5
# speedup vs baseline: 1.1322x; 1.0865x over previous
"""MoE (top-2 of 8 experts, D=H=1024) on 8 Trainium2 NeuronCores.

Strategy (expert-parallel, matching the sharding hint):
  - Host computes the router (softmax + top-k + expert-sort dispatch) in
    float64 -- the dispatch/sharding decision, 0.2% of total FLOPs.
  - Tokens are gathered per expert (capacity-padded); core c gets expert c's
    token block plus expert c's weights.
  - Each core runs the 2-layer expert MLP in "transposed activation" layout
    (activations are [feature, token]) so no on-device transposes are needed.
    Both layers are m-outer: each 128-wide output block accumulates over all
    contraction chunks in a single PSUM bank, then is drained (gelu for layer
    1, fp16 copy + DMA-out for layer 2) while the PE works on the next block.
    Host pre-reorders the weights so each m-block is one contiguous DMA chunk
    and availability tracks the stream.
  - DMA: weights stream on the Sync HWDGE ring in consumption order (FIFO =
    priority); the Scalar ring carries the bias + first w1 chunk up front and
    the output chunks at the end.
  - Host scales rows by the gate probability (zero for padding rows) and
    scatter-adds back into the [T, D] output.
"""

import os
import sys

import numpy as np

for _p in ("/opt/trn_rl_repo", "/root/.axon_site/_ro/trn_rl_repo"):
    if os.path.isdir(_p) and _p not in sys.path:
        sys.path.append(_p)


def _ensure_ntff_hook():
    """Register the axon NTFF profiling hook if the image's antenv lacks it."""
    try:
        import antenv.axon_hooks  # noqa: F401

        return
    except ImportError:
        pass
    import types

    try:
        import antenv
    except ImportError:
        return
    mod = types.ModuleType("antenv.axon_hooks")
    _hook = [None]
    mod.set_axon_ntff_profile_hook = lambda h: _hook.__setitem__(0, h)
    mod.get_axon_ntff_profile_hook = lambda: _hook[0]
    sys.modules["antenv.axon_hooks"] = mod
    antenv.axon_hooks = mod
    try:
        from trn_agent_boot.trn_boot import _ntff_profile_via_ctypes

        mod.set_axon_ntff_profile_hook(
            _ntff_profile_via_ctypes("/opt/axon/libaxon_pjrt.so")
        )
    except Exception:
        pass


_ensure_ntff_hook()

D, H, E, TOPK = 1024, 1024, 8, 2
N_CORES = 8
P = 128  # partitions
KD = D // P
KH = H // P

NWARM = int(os.environ.get("MOE_NWARM", "28"))

_compiled_cache = {}


def _build_program(C):
    """One expert's MLP over a [C] token block; same program on all cores."""
    from concourse import bacc, mybir, tile

    f32 = mybir.dt.float32
    f16 = mybir.dt.float16
    nc = bacc.Bacc(None, target_bir_lowering=False, debug=False)

    # Host-packed layouts: everything contiguous per partition.
    #   xt[p, k, c]    = x[token c, k*128+p]
    #   w1[p, m, k, j] = w_in[k*128+p, m*128+j]
    #   w2[p, m, k, j] = w_out[k*128+p, m*128+j]
    #   b1[p, m]       = b_in[m*128+p]
    #   yT[p, m, c]    = y[token c, m*128+p]
    xt_d = nc.dram_tensor("xt", [P, KD, C], f16, kind="ExternalInput")
    w1_d = nc.dram_tensor("w1", [P, KH, KD, P], f16, kind="ExternalInput")
    w2_d = nc.dram_tensor("w2", [P, KD, KH, P], f16, kind="ExternalInput")
    b1_d = nc.dram_tensor("b1", [P, KH], f32, kind="ExternalInput")
    yT_d = nc.dram_tensor("yT", [P, KD, C], f16, kind="ExternalOutput")

    with tile.TileContext(nc) as tc:
        with (
            tc.tile_pool(name="wpool", bufs=1) as wpool,
            tc.tile_pool(name="xpool", bufs=1) as xpool,
            tc.tile_pool(name="hpool", bufs=1) as hpool,
            tc.tile_pool(name="ypool", bufs=1) as ypool,
            tc.tile_pool(name="bpool", bufs=1) as bpool,
            tc.tile_pool(name="ps1pool", bufs=4, space="PSUM") as ps1pool,
            tc.tile_pool(name="ps2pool", bufs=4, space="PSUM") as ps2pool,
        ):
            w1 = wpool.tile([P, KH, KD, P], f16, tag="w1")
            w2 = wpool.tile([P, KD, KH, P], f16, tag="w2")
            xt = xpool.tile([P, KD, C], f16, tag="xt")
            b1 = bpool.tile([P, KH], f32, tag="b1")
            ht = hpool.tile([P, KH, C], f16, tag="ht")
            yt = ypool.tile([P, KD, C], f16, tag="yt")

            # Sync HWDGE ring, FIFO = priority: strict consumption order.
            # All weight/activation streaming stays on ONE ring so the
            # per-ring round-robin can't starve an urgent chunk behind a
            # bulk one; the scalar ring carries only the (tiny) bias and
            # the output chunks at the end.
            nc.scalar.dma_start(b1[:], b1_d[:])
            nc.sync.dma_start(w1[:, 0:2], w1_d[:, 0:2])
            nc.sync.dma_start(xt[:], xt_d[:])
            nc.sync.dma_start(w1[:, 2:4], w1_d[:, 2:4])
            nc.sync.dma_start(w1[:, 4:6], w1_d[:, 4:6])
            nc.sync.dma_start(w1[:, 6:8], w1_d[:, 6:8])
            nc.sync.dma_start(w2[:, 0:4], w2_d[:, 0:4])
            nc.sync.dma_start(w2[:, 4:8], w2_d[:, 4:8])

            gelu = mybir.ActivationFunctionType.Gelu_apprx_tanh

            # PE warmup during the initial DMA window: dummy matmuls flip the
            # HAM clock gate to 8/8 before the real stream begins.
            wz = bpool.tile([P, P], f16, tag="wz")
            nc.vector.memset(wz[:], 0.0)
            psw = ps2pool.tile([P, 512], f32, tag="ps2", name="ps_warm")
            for i in range(NWARM):
                nc.tensor.matmul(
                    psw[:, :P], wz[:], wz[:], start=(i == 0), stop=(i == NWARM - 1)
                )

            # layer 1, m-outer: one PSUM bank per output block, gelu drains
            # while the PE streams the next block.
            ps1 = [ps1pool.tile([P, 512], f32, tag="ps1", name=f"ps1_{m}") for m in range(KH)]
            for m in range(KH):
                for k in range(KD):
                    nc.tensor.matmul(
                        ps1[m][:, :C],
                        w1[:, m, k, :],
                        xt[:, k, :],
                        start=(k == 0),
                        stop=(k == KD - 1),
                    )
                nc.scalar.activation(
                    ht[:, m, :], ps1[m][:, :C], gelu, bias=b1[:, m : m + 1]
                )

            # layer 2, m-outer: drain each output block to SBUF (fp16) and
            # DMA it out (Scalar ring) while the next block computes.
            ps2 = [ps2pool.tile([P, 512], f32, tag="ps2", name=f"ps2_{m}") for m in range(KD)]
            for m in range(KD):
                for k in range(KH):
                    nc.tensor.matmul(
                        ps2[m][:, :C],
                        w2[:, m, k, :],
                        ht[:, k, :],
                        start=(k == 0),
                        stop=(k == KH - 1),
                    )
                nc.vector.tensor_copy(yt[:, m, :], ps2[m][:, :C])
                if m % 2 == 1:
                    # alternate output pairs between the two HWDGE rings so
                    # the final issue chain is short
                    eng = nc.scalar if (m // 2) % 2 == 0 else nc.sync
                    eng.dma_start(
                        yT_d[:, m - 1 : m + 1, :], yt[:, m - 1 : m + 1, :]
                    )

    nc.compile()
    if not nc.is_finalized():
        nc.finalize()
    return nc


def _get_program(C):
    if C not in _compiled_cache:
        _compiled_cache[C] = _build_program(C)
    return _compiled_cache[C]


def _route(x2, router_w):
    """Host router in float64: top-2 experts + gate probs per token."""
    logits = x2.astype(np.float64) @ np.asarray(router_w, np.float64)
    logits -= logits.max(axis=-1, keepdims=True)
    ex = np.exp(logits)
    probs = ex / ex.sum(axis=-1, keepdims=True)
    top_e = np.argsort(-probs, axis=-1, kind="stable")[:, :TOPK]  # [T, K]
    top_p = np.take_along_axis(probs, top_e, axis=-1)  # [T, K]
    return top_e, top_p.astype(np.float32)


def _pack_w(w):
    """[D, H] -> [P, KH, KD, P] fp16 with w_packed[p, m, k, j] = w[k*128+p, m*128+j]."""
    return np.ascontiguousarray(
        w.reshape(KD, P, KH, P).transpose(1, 2, 0, 3)
    ).astype(np.float16)


def kernel(input_batch, router_w, w_in, b_in, w_out, b_out, run_kwargs=None):
    from concourse.bass_utils import run_bass_kernel_spmd

    x = np.ascontiguousarray(np.asarray(input_batch, np.float32))
    B, S, Dm = x.shape
    T = B * S
    x2 = x.reshape(T, Dm)

    top_e, top_p = _route(x2, router_w)

    # per-expert dispatch lists, in expert-sorted (token, k) order like the
    # reference's stable argsort over flattened (token, k) pairs
    tok_lists = [[] for _ in range(E)]
    p_lists = [[] for _ in range(E)]
    for t in range(T):
        for j in range(TOPK):
            e = top_e[t, j]
            tok_lists[e].append(t)
            p_lists[e].append(top_p[t, j])

    counts = [len(l) for l in tok_lists]
    # capacity per wave; a PSUM bank caps the matmul free dim at 512, so an
    # expert with >512 routed tokens (never happens for the spec'd input
    # distribution) is processed in multiple SPMD waves
    n_waves = max(1, -(-max(counts) // 512))
    if n_waves == 1:
        C = max(256, -(-max(counts) // 8) * 8)  # multiple of 8
    else:
        C = 512

    nc = _get_program(C)

    w_in = np.asarray(w_in, np.float32)
    w_out = np.asarray(w_out, np.float32)
    b_in = np.asarray(b_in, np.float32)
    b_out = np.asarray(b_out, np.float32)

    w1_packed = [_pack_w(w_in[e]) for e in range(E)]
    w2_packed = [_pack_w(w_out[e]) for e in range(E)]
    b1_packed = [
        np.ascontiguousarray(b_in[e].reshape(KH, P).T).astype(np.float32)
        for e in range(E)
    ]

    out = np.zeros((T, Dm), np.float32)
    for w in range(n_waves):
        in_maps = []
        for e in range(E):
            idx = np.asarray(tok_lists[e][w * C : (w + 1) * C], np.int64)
            xt = np.zeros((P, KD, C), np.float16)
            if len(idx):
                # xt[p, k, c] = x2[idx[c], k*128+p]
                xt[:, :, : len(idx)] = (
                    x2[idx].astype(np.float16).T.reshape(KD, P, len(idx)).transpose(1, 0, 2)
                )
            in_maps.append(
                {
                    "xt": xt,
                    "w1": w1_packed[e],
                    "w2": w2_packed[e],
                    "b1": b1_packed[e],
                }
            )

        res = run_bass_kernel_spmd(
            nc, in_maps, core_ids=list(range(N_CORES)), **(run_kwargs or {})
        )
        kernel.last_results = res

        for e in range(E):
            idx = np.asarray(tok_lists[e][w * C : (w + 1) * C], np.int64)
            n = len(idx)
            if n == 0:
                continue
            p = np.asarray(p_lists[e][w * C : (w + 1) * C], np.float32)
            yT = res.results[e]["yT"]  # [P, KD, C] fp16
            y = yT.transpose(2, 1, 0).reshape(C, Dm)[:n].astype(np.float32)
            y = (y + b_out[e]) * p[:, None]
            np.add.at(out, idx, y)

    return out.reshape(B, S, Dm)


# revision 7
# speedup vs baseline: 1.1528x; 1.0182x over previous
"""MoE (top-2 of 8 experts, D=H=1024) on 8 Trainium2 NeuronCores.

Strategy (expert-parallel, matching the sharding hint):
  - Host computes the router (softmax + top-k + expert-sort dispatch) in
    float64 -- the dispatch/sharding decision, 0.2% of total FLOPs.
  - Tokens are gathered per expert (capacity-padded); core c gets expert c's
    token block plus expert c's weights.
  - Each core runs the 2-layer expert MLP in "transposed activation" layout
    (activations are [feature, token]) so no on-device transposes are needed.
    Both layers are m-outer: each 128-wide output block accumulates over all
    contraction chunks in a single PSUM bank, then is drained (gelu for layer
    1, fp16 copy + DMA-out for layer 2) while the PE works on the next block.
    Host pre-reorders the weights so each m-block is one contiguous DMA chunk
    and availability tracks the stream.
  - DMA: weights stream on the Sync HWDGE ring in consumption order (FIFO =
    priority); the Scalar ring carries the bias + first w1 chunk up front and
    the output chunks at the end.
  - Host scales rows by the gate probability (zero for padding rows) and
    scatter-adds back into the [T, D] output.
"""

import os
import sys

import numpy as np

for _p in ("/opt/trn_rl_repo", "/root/.axon_site/_ro/trn_rl_repo"):
    if os.path.isdir(_p) and _p not in sys.path:
        sys.path.append(_p)


def _ensure_ntff_hook():
    """Register the axon NTFF profiling hook if the image's antenv lacks it."""
    try:
        import antenv.axon_hooks  # noqa: F401

        return
    except ImportError:
        pass
    import types

    try:
        import antenv
    except ImportError:
        return
    mod = types.ModuleType("antenv.axon_hooks")
    _hook = [None]
    mod.set_axon_ntff_profile_hook = lambda h: _hook.__setitem__(0, h)
    mod.get_axon_ntff_profile_hook = lambda: _hook[0]
    sys.modules["antenv.axon_hooks"] = mod
    antenv.axon_hooks = mod
    try:
        from trn_agent_boot.trn_boot import _ntff_profile_via_ctypes

        mod.set_axon_ntff_profile_hook(
            _ntff_profile_via_ctypes("/opt/axon/libaxon_pjrt.so")
        )
    except Exception:
        pass


_ensure_ntff_hook()

D, H, E, TOPK = 1024, 1024, 8, 2
N_CORES = 8
P = 128  # partitions
KD = D // P
KH = H // P

NWARM = int(os.environ.get("MOE_NWARM", "40"))

_compiled_cache = {}


def _build_program(C):
    """One expert's MLP over a [C] token block; same program on all cores."""
    from concourse import bacc, mybir, tile

    f32 = mybir.dt.float32
    f16 = mybir.dt.float16
    nc = bacc.Bacc(None, target_bir_lowering=False, debug=False)

    # Host-packed layouts: everything contiguous per partition.
    #   xt[p, k, c]    = x[token c, k*128+p]
    #   w1[p, m, k, j] = w_in[k*128+p, m*128+j]
    #   w2[p, m, k, j] = w_out[k*128+p, m*128+j]
    #   b1[p, m]       = b_in[m*128+p]
    #   yT[p, m, c]    = y[token c, m*128+p]
    xt_d = nc.dram_tensor("xt", [P, KD, C], f16, kind="ExternalInput")
    w1_d = nc.dram_tensor("w1", [P, KH, KD, P], f16, kind="ExternalInput")
    w2_d = nc.dram_tensor("w2", [P, KD, KH, P], f16, kind="ExternalInput")
    b1_d = nc.dram_tensor("b1", [P, KH], f32, kind="ExternalInput")
    yT_d = nc.dram_tensor("yT", [P, KD, C], f16, kind="ExternalOutput")

    with tile.TileContext(nc) as tc:
        with (
            tc.tile_pool(name="wpool", bufs=1) as wpool,
            tc.tile_pool(name="xpool", bufs=1) as xpool,
            tc.tile_pool(name="hpool", bufs=1) as hpool,
            tc.tile_pool(name="ypool", bufs=1) as ypool,
            tc.tile_pool(name="bpool", bufs=1) as bpool,
            tc.tile_pool(name="ps1pool", bufs=4, space="PSUM") as ps1pool,
            tc.tile_pool(name="ps2pool", bufs=4, space="PSUM") as ps2pool,
        ):
            w1 = wpool.tile([P, KH, KD, P], f16, tag="w1")
            w2 = wpool.tile([P, KD, KH, P], f16, tag="w2")
            xt = xpool.tile([P, KD, C], f16, tag="xt")
            b1 = bpool.tile([P, KH], f32, tag="b1")
            ht = hpool.tile([P, KH, C], f16, tag="ht")
            yt = ypool.tile([P, KD, C], f16, tag="yt")

            # Sync HWDGE ring, FIFO = priority: strict consumption order.
            # All weight/activation streaming stays on ONE ring so the
            # per-ring round-robin can't starve an urgent chunk behind a
            # bulk one; the scalar ring carries only the (tiny) bias and
            # the output chunks at the end.
            nc.scalar.dma_start(b1[:], b1_d[:])
            nc.sync.dma_start(xt[:], xt_d[:])
            nc.sync.dma_start(w1[:, 0:1, 0:4], w1_d[:, 0:1, 0:4])
            nc.sync.dma_start(w1[:, 0:1, 4:8], w1_d[:, 0:1, 4:8])
            nc.sync.dma_start(w1[:, 1:2], w1_d[:, 1:2])
            nc.sync.dma_start(w1[:, 2:4], w1_d[:, 2:4])
            nc.sync.dma_start(w1[:, 4:6], w1_d[:, 4:6])
            nc.sync.dma_start(w1[:, 6:8], w1_d[:, 6:8])
            nc.sync.dma_start(w2[:, 0:4], w2_d[:, 0:4])
            nc.sync.dma_start(w2[:, 4:8], w2_d[:, 4:8])

            gelu = mybir.ActivationFunctionType.Gelu_apprx_tanh

            # PE warmup during the initial DMA window: dummy matmuls flip the
            # HAM clock gate to 8/8 before the real stream begins.
            wz = bpool.tile([P, P], f16, tag="wz")
            nc.vector.memset(wz[:], 0.0)
            psw = ps2pool.tile([P, 512], f32, tag="ps2", name="ps_warm")
            for i in range(NWARM):
                nc.tensor.matmul(
                    psw[:, :P], wz[:], wz[:], start=(i == 0), stop=(i == NWARM - 1)
                )

            # layer 1, m-outer: one PSUM bank per output block, gelu drains
            # while the PE streams the next block.
            ps1 = [ps1pool.tile([P, 512], f32, tag="ps1", name=f"ps1_{m}") for m in range(KH)]
            for m in range(KH):
                for k in range(KD):
                    nc.tensor.matmul(
                        ps1[m][:, :C],
                        w1[:, m, k, :],
                        xt[:, k, :],
                        start=(k == 0),
                        stop=(k == KD - 1),
                    )
                nc.scalar.activation(
                    ht[:, m, :], ps1[m][:, :C], gelu, bias=b1[:, m : m + 1]
                )

            # layer 2, m-outer: drain each output block to SBUF (fp16) and
            # DMA it out (Scalar ring) while the next block computes.
            ps2 = [ps2pool.tile([P, 512], f32, tag="ps2", name=f"ps2_{m}") for m in range(KD)]
            for m in range(KD):
                for k in range(KH):
                    nc.tensor.matmul(
                        ps2[m][:, :C],
                        w2[:, m, k, :],
                        ht[:, k, :],
                        start=(k == 0),
                        stop=(k == KH - 1),
                    )
                nc.vector.tensor_copy(yt[:, m, :], ps2[m][:, :C])
                if m % 2 == 1:
                    # alternate output pairs between the two HWDGE rings so
                    # the final issue chain is short
                    eng = nc.scalar if (m // 2) % 2 == 0 else nc.sync
                    eng.dma_start(
                        yT_d[:, m - 1 : m + 1, :], yt[:, m - 1 : m + 1, :]
                    )

    nc.compile()
    if not nc.is_finalized():
        nc.finalize()
    return nc


def _get_program(C):
    if C not in _compiled_cache:
        _compiled_cache[C] = _build_program(C)
    return _compiled_cache[C]


def _route(x2, router_w):
    """Host router in float64: top-2 experts + gate probs per token."""
    logits = x2.astype(np.float64) @ np.asarray(router_w, np.float64)
    logits -= logits.max(axis=-1, keepdims=True)
    ex = np.exp(logits)
    probs = ex / ex.sum(axis=-1, keepdims=True)
    top_e = np.argsort(-probs, axis=-1, kind="stable")[:, :TOPK]  # [T, K]
    top_p = np.take_along_axis(probs, top_e, axis=-1)  # [T, K]
    return top_e, top_p.astype(np.float32)


def _pack_w(w):
    """[D, H] -> [P, KH, KD, P] fp16 with w_packed[p, m, k, j] = w[k*128+p, m*128+j]."""
    return np.ascontiguousarray(
        w.reshape(KD, P, KH, P).transpose(1, 2, 0, 3)
    ).astype(np.float16)


def kernel(input_batch, router_w, w_in, b_in, w_out, b_out, run_kwargs=None):
    from concourse.bass_utils import run_bass_kernel_spmd

    x = np.ascontiguousarray(np.asarray(input_batch, np.float32))
    B, S, Dm = x.shape
    T = B * S
    x2 = x.reshape(T, Dm)

    top_e, top_p = _route(x2, router_w)

    # per-expert dispatch lists, in expert-sorted (token, k) order like the
    # reference's stable argsort over flattened (token, k) pairs
    tok_lists = [[] for _ in range(E)]
    p_lists = [[] for _ in range(E)]
    for t in range(T):
        for j in range(TOPK):
            e = top_e[t, j]
            tok_lists[e].append(t)
            p_lists[e].append(top_p[t, j])

    counts = [len(l) for l in tok_lists]
    # capacity per wave; a PSUM bank caps the matmul free dim at 512, so an
    # expert with >512 routed tokens (never happens for the spec'd input
    # distribution) is processed in multiple SPMD waves
    n_waves = max(1, -(-max(counts) // 512))
    if n_waves == 1:
        C = max(256, -(-max(counts) // 8) * 8)  # multiple of 8
    else:
        C = 512

    nc = _get_program(C)

    w_in = np.asarray(w_in, np.float32)
    w_out = np.asarray(w_out, np.float32)
    b_in = np.asarray(b_in, np.float32)
    b_out = np.asarray(b_out, np.float32)

    w1_packed = [_pack_w(w_in[e]) for e in range(E)]
    w2_packed = [_pack_w(w_out[e]) for e in range(E)]
    b1_packed = [
        np.ascontiguousarray(b_in[e].reshape(KH, P).T).astype(np.float32)
        for e in range(E)
    ]

    out = np.zeros((T, Dm), np.float32)
    for w in range(n_waves):
        in_maps = []
        for e in range(E):
            idx = np.asarray(tok_lists[e][w * C : (w + 1) * C], np.int64)
            xt = np.zeros((P, KD, C), np.float16)
            if len(idx):
                # xt[p, k, c] = x2[idx[c], k*128+p]
                xt[:, :, : len(idx)] = (
                    x2[idx].astype(np.float16).T.reshape(KD, P, len(idx)).transpose(1, 0, 2)
                )
            in_maps.append(
                {
                    "xt": xt,
                    "w1": w1_packed[e],
                    "w2": w2_packed[e],
                    "b1": b1_packed[e],
                }
            )

        res = run_bass_kernel_spmd(
            nc, in_maps, core_ids=list(range(N_CORES)), **(run_kwargs or {})
        )
        kernel.last_results = res

        for e in range(E):
            idx = np.asarray(tok_lists[e][w * C : (w + 1) * C], np.int64)
            n = len(idx)
            if n == 0:
                continue
            p = np.asarray(p_lists[e][w * C : (w + 1) * C], np.float32)
            yT = res.results[e]["yT"]  # [P, KD, C] fp16
            y = yT.transpose(2, 1, 0).reshape(C, Dm)[:n].astype(np.float32)
            y = (y + b_out[e]) * p[:, None]
            np.add.at(out, idx, y)

    return out.reshape(B, S, Dm)


# revision 9
# speedup vs baseline: 1.2679x; 1.0999x over previous
"""MoE (top-2 of 8 experts, D=H=1024) on 8 Trainium2 NeuronCores.

Strategy (expert-parallel, matching the sharding hint):
  - Host computes the router (softmax + top-k + expert-sort dispatch) in
    float64 -- the dispatch/sharding decision, 0.2% of total FLOPs.
  - Tokens are gathered per expert (capacity-padded); core c gets expert c's
    token block plus expert c's weights.
  - Each core runs the 2-layer expert MLP in "transposed activation" layout
    (activations are [feature, token]) so no on-device transposes are needed.
    Both layers are m-outer: each 128-wide output block accumulates over all
    contraction chunks in a single PSUM bank, then is drained (gelu for layer
    1, fp16 copy + DMA-out for layer 2) while the PE works on the next block.
    Host pre-reorders the weights so each m-block is one contiguous DMA chunk
    and availability tracks the stream.
  - DMA: weights stream on the Sync HWDGE ring in consumption order (FIFO =
    priority); the Scalar ring carries the bias + first w1 chunk up front and
    the output chunks at the end.
  - Host scales rows by the gate probability (zero for padding rows) and
    scatter-adds back into the [T, D] output.
"""

import os
import sys

import numpy as np

for _p in ("/opt/trn_rl_repo", "/root/.axon_site/_ro/trn_rl_repo"):
    if os.path.isdir(_p) and _p not in sys.path:
        sys.path.append(_p)


def _ensure_ntff_hook():
    """Register the axon NTFF profiling hook if the image's antenv lacks it."""
    try:
        import antenv.axon_hooks  # noqa: F401

        return
    except ImportError:
        pass
    import types

    try:
        import antenv
    except ImportError:
        return
    mod = types.ModuleType("antenv.axon_hooks")
    _hook = [None]
    mod.set_axon_ntff_profile_hook = lambda h: _hook.__setitem__(0, h)
    mod.get_axon_ntff_profile_hook = lambda: _hook[0]
    sys.modules["antenv.axon_hooks"] = mod
    antenv.axon_hooks = mod
    try:
        from trn_agent_boot.trn_boot import _ntff_profile_via_ctypes

        mod.set_axon_ntff_profile_hook(
            _ntff_profile_via_ctypes("/opt/axon/libaxon_pjrt.so")
        )
    except Exception:
        pass


_ensure_ntff_hook()

D, H, E, TOPK = 1024, 1024, 8, 2
N_CORES = 8
P = 128  # partitions
KD = D // P
KH = H // P

NWARM = int(os.environ.get("MOE_NWARM", "40"))

_compiled_cache = {}


def _build_program(C):
    """One expert's MLP over a [C] token block; same program on all cores."""
    from concourse import bacc, mybir, tile

    f32 = mybir.dt.float32
    f16 = mybir.dt.float16
    nc = bacc.Bacc(None, target_bir_lowering=False, debug=False)

    # Host-packed layouts: everything contiguous per partition.
    #   xt[p, k, c]    = x[token c, k*128+p]
    #   w1[p, m, k, j] = w_in[k*128+p, m*128+j]
    #   w2[p, m, k, j] = w_out[k*128+p, m*128+j]
    #   b1[p, m]       = b_in[m*128+p]
    #   yT[p, m, c]    = y[token c, m*128+p]
    xt_d = nc.dram_tensor("xt", [P, KD, C], f16, kind="ExternalInput")
    w1_d = nc.dram_tensor("w1", [P, KH, KD, P], f16, kind="ExternalInput")
    w2_d = nc.dram_tensor("w2", [P, KD, KH, P], f16, kind="ExternalInput")
    b1_d = nc.dram_tensor("b1", [P, KH], f32, kind="ExternalInput")
    yT_d = nc.dram_tensor("yT", [P, KD, C], f16, kind="ExternalOutput")

    with tile.TileContext(nc) as tc:
        with (
            tc.tile_pool(name="wpool", bufs=1) as wpool,
            tc.tile_pool(name="xpool", bufs=1) as xpool,
            tc.tile_pool(name="hpool", bufs=1) as hpool,
            tc.tile_pool(name="ypool", bufs=1) as ypool,
            tc.tile_pool(name="bpool", bufs=1) as bpool,
            tc.tile_pool(name="ps1pool", bufs=4, space="PSUM") as ps1pool,
            tc.tile_pool(name="ps2pool", bufs=4, space="PSUM") as ps2pool,
        ):
            w1 = wpool.tile([P, KH, KD, P], f16, tag="w1")
            w2 = wpool.tile([P, KD, KH, P], f16, tag="w2")
            xt = xpool.tile([P, KD, C], f16, tag="xt")
            b1 = bpool.tile([P, KH], f32, tag="b1")
            ht = hpool.tile([P, KH, C], f16, tag="ht")
            yt = ypool.tile([P, KD, C], f16, tag="yt")

            # Sync HWDGE ring, FIFO = priority: strict consumption order.
            # All weight/activation streaming stays on ONE ring so the
            # per-ring round-robin can't starve an urgent chunk behind a
            # bulk one; the scalar ring carries only the (tiny) bias and
            # the output chunks at the end.
            nc.scalar.dma_start(b1[:], b1_d[:])
            nc.sync.dma_start(xt[:], xt_d[:])
            # w1 chunk sizes track the consumption rate: halves for the
            # first group (earliest semaphore), then one m-block per DMA.
            nc.sync.dma_start(w1[:, 0:1, 0:4], w1_d[:, 0:1, 0:4])
            nc.sync.dma_start(w1[:, 0:1, 4:8], w1_d[:, 0:1, 4:8])
            for m in range(1, KH):
                nc.sync.dma_start(w1[:, m : m + 1], w1_d[:, m : m + 1])
            for m in range(0, KD, 2):
                nc.sync.dma_start(w2[:, m : m + 2], w2_d[:, m : m + 2])

            gelu = mybir.ActivationFunctionType.Gelu_apprx_tanh

            # PE warmup during the initial DMA window: dummy matmuls flip the
            # HAM clock gate to 8/8 before the real stream begins.
            wz = bpool.tile([P, P], f16, tag="wz")
            nc.vector.memset(wz[:], 0.0)
            psw = ps2pool.tile([P, 512], f32, tag="ps2", name="ps_warm")
            for i in range(NWARM):
                nc.tensor.matmul(
                    psw[:, :P], wz[:], wz[:], start=(i == 0), stop=(i == NWARM - 1)
                )

            # layer 1, m-outer: one PSUM bank per output block, gelu drains
            # while the PE streams the next block.
            ps1 = [ps1pool.tile([P, 512], f32, tag="ps1", name=f"ps1_{m}") for m in range(KH)]
            for m in range(KH):
                for k in range(KD):
                    nc.tensor.matmul(
                        ps1[m][:, :C],
                        w1[:, m, k, :],
                        xt[:, k, :],
                        start=(k == 0),
                        stop=(k == KD - 1),
                    )
                nc.scalar.activation(
                    ht[:, m, :], ps1[m][:, :C], gelu, bias=b1[:, m : m + 1]
                )

            # layer 2, m-outer: drain each output block to SBUF (fp16) and
            # DMA it out (Scalar ring) while the next block computes.
            ps2 = [ps2pool.tile([P, 512], f32, tag="ps2", name=f"ps2_{m}") for m in range(KD)]
            for m in range(KD):
                for k in range(KH):
                    nc.tensor.matmul(
                        ps2[m][:, :C],
                        w2[:, m, k, :],
                        ht[:, k, :],
                        start=(k == 0),
                        stop=(k == KH - 1),
                    )
                nc.vector.tensor_copy(yt[:, m, :], ps2[m][:, :C])
                # output chunks alternate between the two HWDGE rings; the
                # last two go out as singles so the final (critical-path)
                # transfer is as small and early as possible
                if m == 1 or m == 3 or m == 5:
                    eng = nc.scalar if m == 3 else nc.sync
                    eng.dma_start(
                        yT_d[:, m - 1 : m + 1, :], yt[:, m - 1 : m + 1, :]
                    )
                elif m >= 6:
                    eng = nc.sync if m == 6 else nc.scalar
                    eng.dma_start(yT_d[:, m : m + 1, :], yt[:, m : m + 1, :])

    nc.compile()
    if not nc.is_finalized():
        nc.finalize()
    return nc


def _get_program(C):
    if C not in _compiled_cache:
        _compiled_cache[C] = _build_program(C)
    return _compiled_cache[C]


def _route(x2, router_w):
    """Host router in float64: top-2 experts + gate probs per token."""
    logits = x2.astype(np.float64) @ np.asarray(router_w, np.float64)
    logits -= logits.max(axis=-1, keepdims=True)
    ex = np.exp(logits)
    probs = ex / ex.sum(axis=-1, keepdims=True)
    top_e = np.argsort(-probs, axis=-1, kind="stable")[:, :TOPK]  # [T, K]
    top_p = np.take_along_axis(probs, top_e, axis=-1)  # [T, K]
    return top_e, top_p.astype(np.float32)


def _pack_w(w):
    """[D, H] -> [P, KH, KD, P] fp16 with w_packed[p, m, k, j] = w[k*128+p, m*128+j]."""
    return np.ascontiguousarray(
        w.reshape(KD, P, KH, P).transpose(1, 2, 0, 3)
    ).astype(np.float16)


def kernel(input_batch, router_w, w_in, b_in, w_out, b_out, run_kwargs=None):
    from concourse.bass_utils import run_bass_kernel_spmd

    x = np.ascontiguousarray(np.asarray(input_batch, np.float32))
    B, S, Dm = x.shape
    T = B * S
    x2 = x.reshape(T, Dm)

    top_e, top_p = _route(x2, router_w)

    # per-expert dispatch lists, in expert-sorted (token, k) order like the
    # reference's stable argsort over flattened (token, k) pairs
    tok_lists = [[] for _ in range(E)]
    p_lists = [[] for _ in range(E)]
    for t in range(T):
        for j in range(TOPK):
            e = top_e[t, j]
            tok_lists[e].append(t)
            p_lists[e].append(top_p[t, j])

    counts = [len(l) for l in tok_lists]
    # capacity per wave; a PSUM bank caps the matmul free dim at 512, so an
    # expert with >512 routed tokens (never happens for the spec'd input
    # distribution) is processed in multiple SPMD waves
    n_waves = max(1, -(-max(counts) // 512))
    if n_waves == 1:
        C = max(256, -(-max(counts) // 8) * 8)  # multiple of 8
    else:
        C = 512

    nc = _get_program(C)

    w_in = np.asarray(w_in, np.float32)
    w_out = np.asarray(w_out, np.float32)
    b_in = np.asarray(b_in, np.float32)
    b_out = np.asarray(b_out, np.float32)

    w1_packed = [_pack_w(w_in[e]) for e in range(E)]
    w2_packed = [_pack_w(w_out[e]) for e in range(E)]
    b1_packed = [
        np.ascontiguousarray(b_in[e].reshape(KH, P).T).astype(np.float32)
        for e in range(E)
    ]

    out = np.zeros((T, Dm), np.float32)
    for w in range(n_waves):
        in_maps = []
        for e in range(E):
            idx = np.asarray(tok_lists[e][w * C : (w + 1) * C], np.int64)
            xt = np.zeros((P, KD, C), np.float16)
            if len(idx):
                # xt[p, k, c] = x2[idx[c], k*128+p]
                xt[:, :, : len(idx)] = (
                    x2[idx].astype(np.float16).T.reshape(KD, P, len(idx)).transpose(1, 0, 2)
                )
            in_maps.append(
                {
                    "xt": xt,
                    "w1": w1_packed[e],
                    "w2": w2_packed[e],
                    "b1": b1_packed[e],
                }
            )

        res = run_bass_kernel_spmd(
            nc, in_maps, core_ids=list(range(N_CORES)), **(run_kwargs or {})
        )
        kernel.last_results = res

        for e in range(E):
            idx = np.asarray(tok_lists[e][w * C : (w + 1) * C], np.int64)
            n = len(idx)
            if n == 0:
                continue
            p = np.asarray(p_lists[e][w * C : (w + 1) * C], np.float32)
            yT = res.results[e]["yT"]  # [P, KD, C] fp16
            y = yT.transpose(2, 1, 0).reshape(C, Dm)[:n].astype(np.float32)
            y = (y + b_out[e]) * p[:, None]
            np.add.at(out, idx, y)

    return out.reshape(B, S, Dm)
